# revision 18
# baseline (speedup 1.0000x reference)
import os
import sys

sys.path.insert(0, "/opt/trn_rl_repo")
os.environ.setdefault("JAX_PLATFORMS", "")

import numpy as np
import ml_dtypes

import concourse.bass as bass
import concourse.bacc as bacc
import concourse.mybir as mybir
import concourse.tile as tile

F32 = mybir.dt.float32
BF16 = mybir.dt.bfloat16
FP8 = mybir.dt.float8e4
AF = mybir.ActivationFunctionType
OP = mybir.AluOpType
DR = mybir.MatmulPerfMode.DoubleRow

B, N, D, S, HW = 2, 4096, 192, 16, 64
RD = D * S  # 3072
NT = 24  # channel tiles of 128
ROWS = 20  # slab rows per core (16 own + halo)
NL = ROWS * HW  # 1280 sites per core
SLAB0 = [0, 14, 30, 44]
OWN0 = [0, 2, 2, 4]

NF8 = np.dtype(ml_dtypes.float8_e4m3)
NBF = np.dtype(ml_dtypes.bfloat16)

_CACHE = {}
LAST = None


def _register_ntff_hook():
    """Register the axon NTFF profile hook if the image didn't inject it.

    concourse.bass_utils reads antenv.axon_hooks.get_axon_ntff_profile_hook()
    when trace=True under axon; this image's antenv lacks that module, so
    build the same ctypes hook trn_agent_boot would have registered.
    """
    import types
    import ctypes
    import contextlib

    if "antenv.axon_hooks" in sys.modules:
        return True
    try:
        import antenv
    except ImportError:
        return False
    so_path = "/opt/axon/libaxon_pjrt.so"
    if not os.path.exists(so_path):
        return False
    try:
        lib = ctypes.CDLL(so_path)
    except OSError:
        return False
    if not hasattr(lib, "axon_start_nrt_profile"):
        return False
    lib.axon_start_nrt_profile.argtypes = [
        ctypes.POINTER(ctypes.c_int64),
        ctypes.c_size_t,
    ]
    lib.axon_start_nrt_profile.restype = ctypes.c_int64
    lib.axon_stop_nrt_profile.argtypes = [ctypes.c_char_p]
    lib.axon_stop_nrt_profile.restype = ctypes.c_int64

    @contextlib.contextmanager
    def _hook(output_dir, device_ids):
        import jax

        jax.devices()
        if device_ids:
            ids = (ctypes.c_int64 * len(device_ids))(*device_ids)
            rc = lib.axon_start_nrt_profile(ids, len(device_ids))
        else:
            rc = lib.axon_start_nrt_profile(None, 0)
        if rc != 0:
            raise RuntimeError(f"axon_start_nrt_profile rc={rc}")
        try:
            yield
        finally:
            n = lib.axon_stop_nrt_profile(str(output_dir).encode())
            if n < 0:
                raise RuntimeError(f"axon_stop_nrt_profile rc={n}")

    mod = types.ModuleType("antenv.axon_hooks")
    _store = {"h": _hook}
    mod.set_axon_ntff_profile_hook = lambda h: _store.__setitem__("h", h)
    mod.get_axon_ntff_profile_hook = lambda: _store["h"]
    sys.modules["antenv.axon_hooks"] = mod
    antenv.axon_hooks = mod
    return True


def _softplus_np(v):
    return np.logaddexp(0.0, v)


def _build(K: int, inv_g: float, inv_p: float, sh: float, fast5: bool):
    dt = 1.0 / K if K > 0 else 1.0
    opt = fast5 and K == 2  # shrinking update regions + 5-point conv
    nc = bacc.Bacc(None, target_bir_lowering=False, debug=False)

    xcm_d = nc.dram_tensor("xcm", [D, NL], F32, kind="ExternalInput")
    h0b_d = nc.dram_tensor("h0b", [RD, NL], BF16, kind="ExternalInput")
    hf80_d = nc.dram_tensor("hf80", [RD, NL], FP8, kind="ExternalInput")
    p1a_d = nc.dram_tensor("p1a", [RD, NL], BF16, kind="ExternalInput")
    p1b_d = nc.dram_tensor("p1b", [RD, NL], BF16, kind="ExternalInput")
    ddb_d = nc.dram_tensor("ddb", [RD, NL], BF16, kind="ExternalInput")
    u1b_d = nc.dram_tensor("u1b", [RD, NL], BF16, kind="ExternalInput")
    cmb_d = nc.dram_tensor("cmb", [128, NL], BF16, kind="ExternalInput")
    dparam_d = nc.dram_tensor("dparam", [D, 1], F32, kind="ExternalInput")
    bg_d = nc.dram_tensor("bg", [RD, 1], F32, kind="ExternalInput")
    w9_d = nc.dram_tensor("w9", [RD, 9], F32, kind="ExternalInput")
    cb5_d = nc.dram_tensor("cb5", [RD, 1], F32, kind="ExternalInput")
    bd5_d = nc.dram_tensor("bd5", [RD, 1], F32, kind="ExternalInput")
    wg8_d = nc.dram_tensor("wg8", [RD, RD], FP8, kind="ExternalInput")
    wp8_d = nc.dram_tensor("wp8", [RD, RD], FP8, kind="ExternalInput")
    sely_d = nc.dram_tensor("sely", [128, NT * 128], BF16, kind="ExternalInput")
    y_d = nc.dram_tensor("y", [D, NL], F32, kind="ExternalOutput")

    NK2 = NT // 2  # DoubleRow k-pairs

    def chunks(ne):
        out, n0 = [], 0
        while n0 < ne:
            out.append((n0, min(512, ne - n0)))
            n0 += 512
        return out

    if opt:
        NE_S = [1216, 1152]  # rows 0..18 after step 1, rows 0..17 after step 2
        RE_S = [19, 18]
        NE_F = 1152
    else:
        NE_S = [NL] * max(K, 1)
        RE_S = [ROWS] * max(K, 1)
        NE_F = NL

    with tile.TileContext(nc) as tc:
        with tc.tile_pool(name="const", bufs=1) as const, \
             tc.tile_pool(name="hp", bufs=1) as hp, \
             tc.tile_pool(name="wsl", bufs=2) as wsl, \
             tc.tile_pool(name="bst", bufs=2) as bst, \
             tc.tile_pool(name="ust", bufs=2) as ust, \
             tc.tile_pool(name="work", bufs=1) as work, \
             tc.tile_pool(name="wk2", bufs=2) as wk2, \
             tc.tile_pool(name="psum", bufs=1, space="PSUM") as psum, \
             tc.tile_pool(name="ps2", bufs=2, space="PSUM") as ps2:

            # ---- persistent state (streamed in up front) ----
            hst = hp.tile([128, NT, NL], BF16, tag="hst")
            hf8 = [hp.tile([128, NT, NL], FP8, tag=f"hf8{i}", name=f"hf8{i}")
                   for i in range(2)]
            if K > 0:
                nc.sync.dma_start(hf8[0][:],
                                  hf80_d[:].rearrange("(t p) n -> p t n", p=128))
            nc.scalar.dma_start(hst[:], h0b_d[:].rearrange("(t p) n -> p t n", p=128))

            # ---- constants ----
            dpA = const.tile([128, 1], F32, tag="dpA")
            dpB = const.tile([64, 1], F32, tag="dpB")
            nc.sync.dma_start(dpA[:], dparam_d[0:128, :])
            nc.sync.dma_start(dpB[:], dparam_d[128:192, :])
            bg_sb = const.tile([128, NT], F32, tag="bg")
            nc.sync.dma_start(bg_sb[:].rearrange("p (t o) -> p t o", o=1),
                              bg_d[:].rearrange("(t p) o -> p t o", p=128))
            if opt:
                cb5_sb = const.tile([128, NT], F32, tag="cb5")
                nc.sync.dma_start(cb5_sb[:].rearrange("p (t o) -> p t o", o=1),
                                  cb5_d[:].rearrange("(t p) o -> p t o", p=128))
                bd5_sb = const.tile([128, NT], F32, tag="bd5")
                nc.sync.dma_start(bd5_sb[:].rearrange("p (t o) -> p t o", o=1),
                                  bd5_d[:].rearrange("(t p) o -> p t o", p=128))
            else:
                w9_sb = const.tile([128, NT, 9], F32, tag="w9")
                nc.sync.dma_start(w9_sb[:], w9_d[:].rearrange("(t p) j -> p t j", p=128))
            sely = const.tile([128, NT * 128], BF16, tag="sely")
            nc.sync.dma_start(sely[:], sely_d[:])
            cmb_sb = const.tile([128, NL], BF16, tag="cmb")
            nc.sync.dma_start(cmb_sb[:], cmb_d[:])

            # ---- K integration steps ----
            for s in range(K):
                cur = hf8[s % 2]
                nxt = hf8[(s + 1) % 2]
                p1_d = p1a_d if s == 0 else p1b_d
                last = s == K - 1
                ne = NE_S[s]
                re = RE_S[s]
                nsp = chunks(ne)
                for rt in range(NT):
                    r0 = 128 * rt
                    wgt = wsl.tile([128, NT * 128], FP8, tag="wgt")
                    wpt = wsl.tile([128, NT * 128], FP8, tag="wpt")
                    nc.sync.dma_start(wgt[:], wg8_d[r0:r0 + 128, :])
                    nc.sync.dma_start(wpt[:], wp8_d[r0:r0 + 128, :])
                    wgt3 = wgt[:].rearrange("p (t m) -> p t m", m=128)
                    wpt3 = wpt[:].rearrange("p (t m) -> p t m", m=128)
                    p1t = bst.tile([128, NL], BF16, tag="p1t")
                    ddbt = bst.tile([128, NL], BF16, tag="ddbt")
                    nc.sync.dma_start(p1t[:, 0:ne], p1_d[r0:r0 + 128, 0:ne])
                    nc.sync.dma_start(ddbt[:, 0:ne], ddb_d[r0:r0 + 128, 0:ne])
                    if s > 0:
                        u1t = ust.tile([128, NL], BF16, tag="u1t")
                        nc.sync.dma_start(u1t[:, 0:ne], u1b_d[r0:r0 + 128, 0:ne])

                    tmp = wk2.tile([128, NL], F32, tag="tmp")
                    dh = work.tile([128, NL], BF16, tag="dh")

                    # f1 seed (+ state): tmp = hst * (dtA*dsb + 1)  [p1 host-folded]
                    nc.vector.tensor_tensor(tmp[:, 0:ne], hst[:, rt, 0:ne],
                                            p1t[:, 0:ne], OP.mult)
                    if s > 0:
                        nc.gpsimd.tensor_tensor(tmp[:, 0:ne], tmp[:, 0:ne],
                                                u1t[:, 0:ne], OP.add)

                    # gate matmuls (fp8 DoubleRow over 12 k-pairs)
                    pgs = [ps2.tile([128, 512], F32, tag="pg0", name="pg0"),
                           ps2.tile([128, 512], F32, tag="pg1", name="pg1"),
                           psum.tile([128, 512], F32, tag="pg2", name="pg2")]
                    pps = [psum.tile([128, 512], F32, tag=f"pp{j}", name=f"pp{j}")
                           for j in range(3)]
                    for kk in range(NK2):
                        for j, (n0, nw) in enumerate(nsp):
                            nc.tensor.matmul(pgs[j][:, 0:nw],
                                             wgt3[:, 2 * kk:2 * kk + 2, :],
                                             cur[:, 2 * kk:2 * kk + 2, n0:n0 + nw],
                                             start=(kk == 0), stop=(kk == NK2 - 1),
                                             perf_mode=DR)

                    hv = hst[:, rt, :].rearrange("p (r c) -> p r c", c=HW)
                    dv = dh[:].rearrange("p (r c) -> p r c", c=HW)
                    if opt:
                        # 5-point stencil: dh = (N+S+E+W) + (c/b)*C; b*dt folded
                        # into the f2 product below.
                        nc.vector.tensor_tensor(dv[:, 1:re, :], hv[:, 0:re - 1, :],
                                                hv[:, 2:re + 1, :], OP.add)
                        nc.vector.tensor_tensor(dv[:, 0:1, :], hv[:, 0:1, :],
                                                hv[:, 1:2, :], OP.add)
                        nc.gpsimd.tensor_tensor(dv[:, 0:re, 1:HW], dv[:, 0:re, 1:HW],
                                                hv[:, 0:re, 0:HW - 1], OP.add)
                        nc.gpsimd.tensor_tensor(dv[:, 0:re, 0:1], dv[:, 0:re, 0:1],
                                                hv[:, 0:re, 0:1], OP.add)
                        nc.gpsimd.tensor_tensor(dv[:, 0:re, 0:HW - 1],
                                                dv[:, 0:re, 0:HW - 1],
                                                hv[:, 0:re, 1:HW], OP.add)
                        nc.gpsimd.tensor_tensor(dv[:, 0:re, HW - 1:HW],
                                                dv[:, 0:re, HW - 1:HW],
                                                hv[:, 0:re, HW - 1:HW], OP.add)
                        nc.vector.scalar_tensor_tensor(dh[:, 0:ne], hst[:, rt, 0:ne],
                                                       cb5_sb[:, rt:rt + 1],
                                                       dh[:, 0:ne], OP.mult, OP.add)
                        # f2 = (dh * b * dt) * ddb
                        nc.vector.scalar_tensor_tensor(
                            dh[:, 0:ne], dh[:, 0:ne], bd5_sb[:, rt:rt + 1],
                            ddbt[:, 0:ne], OP.mult, OP.mult)
                    else:
                        # general depthwise 3x3 (dt folded into w9)
                        def segs(dd, n):
                            if dd == 0:
                                return [((0, n), (0, n))]
                            if dd == -1:
                                return [((1, n - 1), (0, n - 1)), ((0, 1), (0, 1))]
                            return [((0, n - 1), (1, n - 1)),
                                    ((n - 1, 1), (n - 1, 1))]

                        first = True
                        for di in (-1, 0, 1):
                            for dj in (-1, 0, 1):
                                idx = 3 * (di + 1) + (dj + 1)
                                w_s = w9_sb[:, rt, idx:idx + 1]
                                for (ro, rn), (ri, _) in segs(di, ROWS):
                                    for (co, cn), (ci, _) in segs(dj, HW):
                                        o = dv[:, ro:ro + rn, co:co + cn]
                                        i_ = hv[:, ri:ri + rn, ci:ci + cn]
                                        if first:
                                            nc.vector.tensor_scalar_mul(o, i_, w_s)
                                        else:
                                            nc.vector.scalar_tensor_tensor(
                                                o, i_, w_s, o, OP.mult, OP.add)
                                first = False
                        nc.vector.tensor_tensor(dh[:, 0:ne], dh[:, 0:ne],
                                                ddbt[:, 0:ne], OP.mult)

                    # sigmoid gate (descaled), overlaps the proj matmuls below
                    gates = []
                    for j, (n0, nw) in enumerate(nsp):
                        g = work.tile([128, 512], BF16, tag=f"gate{j}", name=f"gate{j}")
                        nc.scalar.activation(g[:, 0:nw], pgs[j][:, 0:nw], AF.Sigmoid,
                                             bias=bg_sb[:, rt:rt + 1], scale=inv_g)
                        gates.append(g)

                    # proj matmuls
                    for kk in range(NK2):
                        for j, (n0, nw) in enumerate(nsp):
                            nc.tensor.matmul(pps[j][:, 0:nw],
                                             wpt3[:, 2 * kk:2 * kk + 2, :],
                                             cur[:, 2 * kk:2 * kk + 2, n0:n0 + nw],
                                             start=(kk == 0), stop=(kk == NK2 - 1),
                                             perf_mode=DR)

                    # f3 = gate * proj (descaled, dt folded); tmp += f3; tmp += dh
                    reacts = []
                    for j, (n0, nw) in enumerate(nsp):
                        rc = work.tile([128, 512], BF16, tag=f"react{j}",
                                       name=f"react{j}")
                        nc.scalar.activation(rc[:, 0:nw], pps[j][:, 0:nw], AF.Copy,
                                             scale=dt * inv_p)
                        reacts.append(rc)
                    for j, (n0, nw) in enumerate(nsp):
                        f3c = work.tile([128, 512], F32, tag="f3c")
                        nc.vector.tensor_tensor(f3c[:, 0:nw], reacts[j][:, 0:nw],
                                                gates[j][:, 0:nw], OP.mult)
                        nc.vector.tensor_tensor(tmp[:, n0:n0 + nw], tmp[:, n0:n0 + nw],
                                                f3c[:, 0:nw], OP.add)
                    # final accumulate writes the bf16 state directly
                    nc.gpsimd.tensor_tensor(hst[:, rt, 0:ne], tmp[:, 0:ne],
                                            dh[:, 0:ne], OP.add)
                    if not last:
                        nc.scalar.activation(nxt[:, rt, 0:ne], hst[:, rt, 0:ne],
                                             AF.Copy, scale=sh)

            # ---- final: y = sum_s h*Cm_bc + x*Dp ----
            nspf = chunks(NE_F)
            pys = [ps2.tile([128, 512], F32, tag="pg0", name="py0"),
                   ps2.tile([128, 512], F32, tag="pg1", name="py1"),
                   psum.tile([128, 512], F32, tag="pg2", name="py2")]
            pyB = [psum.tile([128, 512], F32, tag=f"pp{j}", name=f"pyB{j}")
                   for j in range(3)]
            for rt in range(NT):
                z = work.tile([128, NL], BF16, tag="dh")
                for j, (n0, nw) in enumerate(nspf):
                    nc.vector.tensor_tensor(z[:, n0:n0 + nw], hst[:, rt, n0:n0 + nw],
                                            cmb_sb[:, n0:n0 + nw], OP.mult)
                bank = pys if rt < 16 else pyB
                st = rt == 0 or rt == 16
                sp_ = rt == 15 or rt == NT - 1
                for j, (n0, nw) in enumerate(nspf):
                    nc.tensor.matmul(bank[j][:, 0:nw], sely[:, 128 * rt:128 * rt + 128],
                                     z[:, n0:n0 + nw], start=st, stop=sp_)
            for j, (n0, nw) in enumerate(nspf):
                xfA = work.tile([128, 512], F32, tag="f3c", name="xfA")
                nc.sync.dma_start(xfA[:, 0:nw], xcm_d[0:128, n0:n0 + nw])
                yA = work.tile([128, 512], F32, tag="yA", name=f"yA{j}")
                nc.vector.scalar_tensor_tensor(yA[:, 0:nw], xfA[:, 0:nw], dpA[:],
                                               pys[j][:, 0:nw], OP.mult, OP.add)
                nc.sync.dma_start(y_d[0:128, n0:n0 + nw], yA[:, 0:nw])
                xfB = work.tile([64, 512], F32, tag="xfB")
                nc.sync.dma_start(xfB[:, 0:nw], xcm_d[128:192, n0:n0 + nw])
                yB = work.tile([64, 512], F32, tag="yB")
                nc.vector.scalar_tensor_tensor(yB[:, 0:nw], xfB[:, 0:nw], dpB[:],
                                               pyB[j][0:64, 0:nw], OP.mult, OP.add)
                nc.sync.dma_start(y_d[128:192, n0:n0 + nw], yB[:, 0:nw])

    nc.compile()
    return nc


def _pow2_scale(target, amax):
    if amax <= 0:
        return 1.0
    return float(2.0 ** np.floor(np.log2(target / amax)))


def _prep(x, dt_self_W, dt_self_b, dt_diff_W, dt_diff_b, B_proj_W, C_proj_W,
          D_param, A_log, diff_conv_w, react_gate_W, react_gate_b,
          react_proj_W, dt):
    A = -_softplus_np(np.asarray(A_log, np.float32))          # (D, S)
    dtA1 = (dt * (A + 1.0)).reshape(RD, 1).astype(np.float32)
    dtA2 = (dt * A).reshape(RD, 1).astype(np.float32)
    w33 = np.asarray(diff_conv_w, np.float32)[:, 0]           # (D, 3, 3)
    w9 = (dt * w33).reshape(D, 1, 9)
    w9 = np.broadcast_to(w9, (D, S, 9)).reshape(RD, 9).astype(np.float32)
    w9f = (dt * w33[:, ::-1, :]).reshape(D, 1, 9)             # vertically flipped
    w9f = np.broadcast_to(w9f, (D, S, 9)).reshape(RD, 9).astype(np.float32)

    # 5-point stencil detection: corners zero, N==S==E==W per channel
    b5 = w33[:, 0, 1]
    fast5 = bool(
        np.all(w33[:, [0, 0, 2, 2], [0, 2, 0, 2]] == 0.0)
        and np.all(np.abs(w33[:, 1, 0] - b5) <= 1e-12)
        and np.all(np.abs(w33[:, 1, 2] - b5) <= 1e-12)
        and np.all(np.abs(w33[:, 2, 1] - b5) <= 1e-12)
        and np.all(np.abs(b5) > 1e-30)
    )
    if fast5:
        cb5 = (w33[:, 1, 1] / b5).astype(np.float32)
        bd5 = (dt * b5).astype(np.float32)
    else:
        cb5 = np.zeros(D, np.float32)
        bd5 = np.zeros(D, np.float32)
    cb5 = np.broadcast_to(cb5[:, None], (D, S)).reshape(RD, 1).copy()
    bd5 = np.broadcast_to(bd5[:, None], (D, S)).reshape(RD, 1).copy()

    WgT = np.ascontiguousarray(np.asarray(react_gate_W, np.float32).T)
    WpT = np.ascontiguousarray(np.asarray(react_proj_W, np.float32).T)
    sg = _pow2_scale(200.0, np.abs(WgT).max())
    sp = _pow2_scale(200.0, np.abs(WpT).max())

    x = np.asarray(x, np.float32)
    Bm = x @ np.asarray(B_proj_W, np.float32).T               # (B, N, S)
    Cm = x @ np.asarray(C_proj_W, np.float32).T               # (B, N, S)
    d_self = np.minimum(
        _softplus_np(x @ np.asarray(dt_self_W, np.float32).T
                     + np.asarray(dt_self_b, np.float32)), 0.15)
    d_diff = np.minimum(
        _softplus_np(x @ np.asarray(dt_diff_W, np.float32).T
                     + np.asarray(dt_diff_b, np.float32)), 0.15)
    maxh0 = (np.abs(x).max(-1) * np.abs(Bm).max(-1)).max()
    sh = _pow2_scale(200.0, 2.2 * maxh0)

    def tilemajor(WT, sc):
        a = WT.reshape(NT, 128, NT, 128).transpose(2, 1, 0, 3).reshape(RD, RD)
        return np.clip(a * sc, -240.0, 240.0).astype(NF8)

    sely = np.zeros((128, NT * 128), np.float32)
    for t in range(NT):
        for p in range(128):
            m = 8 * t + p // 16 if t < 16 else 8 * (t - 16) + p // 16
            sely[p, 128 * t + m] = 1.0

    shared = dict(
        dparam=np.asarray(D_param, np.float32).reshape(D, 1),
        bg=np.asarray(react_gate_b, np.float32).reshape(RD, 1),
        cb5=cb5,
        bd5=bd5,
        wg8=tilemajor(WgT, sg),
        wp8=tilemajor(WpT, sp),
        sely=sely.astype(NBF),
    )
    fields = dict(Bm=Bm, Cm=Cm, d_self=d_self, d_diff=d_diff,
                  dtA1=dtA1, dtA2=dtA2)
    return shared, fields, w9, w9f, sg, sp, sh, fast5


def kernel(x, dt_self_W, dt_self_b, dt_diff_W, dt_diff_b, B_proj_W, C_proj_W,
           D_param, A_log, diff_conv_w, react_gate_W, react_gate_b,
           react_proj_W, K_steps):
    from concourse.bass_utils import run_bass_kernel_spmd

    K = int(np.asarray(K_steps).item())
    dt = 1.0 / K if K > 0 else 1.0

    x = np.asarray(x, np.float32)
    shared, fields, w9, w9f, sg, sp, sh, fast5 = _prep(
        x, dt_self_W, dt_self_b, dt_diff_W, dt_diff_b, B_proj_W, C_proj_W,
        D_param, A_log, diff_conv_w, react_gate_W, react_gate_b,
        react_proj_W, dt)
    key = (K, sg, sp, sh, fast5)
    if key not in _CACHE:
        _CACHE[key] = _build(K, 1.0 / (sg * sh), 1.0 / (sp * sh), sh, fast5)
    nc = _CACHE[key]

    xg = x.reshape(B, HW, HW, D)
    Bg = fields["Bm"].reshape(B, HW, HW, S)
    Cg = fields["Cm"].reshape(B, HW, HW, S)
    dsg = fields["d_self"].reshape(B, HW, HW, D).astype(np.float32)
    ddg = fields["d_diff"].reshape(B, HW, HW, D).astype(np.float32)
    dtA1 = fields["dtA1"]
    dtA2 = fields["dtA2"]
    in_maps = []
    for core in range(8):
        b, rb = core // 4, core % 4
        if rb == 3:
            sl = np.s_[63:43:-1]  # reversed slab, own at rows 0..15
            w9c = w9f
        else:
            sl = np.s_[SLAB0[rb]:SLAB0[rb] + ROWS]
            w9c = w9
        slab = xg[b, sl].reshape(NL, D)
        bslab = np.asarray(Bg[b, sl], np.float32).reshape(NL, S)
        cslab = np.asarray(Cg[b, sl], np.float32).reshape(NL, S)
        dss = dsg[b, sl].reshape(NL, D)
        dds = ddg[b, sl].reshape(NL, D)
        h0 = np.ascontiguousarray(
            (slab[:, :, None] * bslab[:, None, :]).reshape(NL, RD).T)  # [RD, NL]
        dsb = np.ascontiguousarray(np.repeat(dss.T, S, axis=0))        # [RD, NL]
        ddb = np.ascontiguousarray(np.repeat(dds.T, S, axis=0))
        u1 = dt * dsb * h0
        p1a = dtA1 * dsb + 1.0
        p1b = dtA2 * dsb + 1.0
        in_maps.append(dict(
            shared,
            xcm=np.ascontiguousarray(slab.T),
            w9=w9c,
            h0b=h0.astype(NBF),
            hf80=np.clip(h0 * sh, -240.0, 240.0).astype(NF8),
            p1a=p1a.astype(NBF),
            p1b=p1b.astype(NBF),
            ddb=ddb.astype(NBF),
            u1b=u1.astype(NBF),
            cmb=np.ascontiguousarray(np.tile(cslab.T, (8, 1))).astype(NBF),
        ))

    trace_ok = False
    try:
        trace_ok = _register_ntff_hook()
    except Exception:
        trace_ok = False
    if trace_ok:
        try:
            r = run_bass_kernel_spmd(nc, in_maps, list(range(8)), trace=True)
        except Exception:
            r = run_bass_kernel_spmd(nc, in_maps, list(range(8)))
    else:
        r = run_bass_kernel_spmd(nc, in_maps, list(range(8)))
    global LAST
    LAST = r
    res = r.results
    y = np.empty((B, N, D), np.float32)
    for core in range(8):
        b, rb = core // 4, core % 4
        yc = res[core]["y"]
        if rb == 3:
            blk = yc.reshape(D, ROWS, HW)[:, 15::-1, :].reshape(D, 1024)
            y[b, 3 * 1024:4 * 1024, :] = blk.T
        else:
            o = OWN0[rb] * HW
            y[b, rb * 1024:(rb + 1) * 1024, :] = yc[:, o:o + 1024].T
    return y


# revision 19
# speedup vs baseline: 1.2189x; 1.2189x over previous
import os
import sys

sys.path.insert(0, "/opt/trn_rl_repo")
os.environ.setdefault("JAX_PLATFORMS", "")

import numpy as np
import ml_dtypes

import concourse.bass as bass
import concourse.bacc as bacc
import concourse.mybir as mybir
import concourse.tile as tile

F32 = mybir.dt.float32
BF16 = mybir.dt.bfloat16
FP8 = mybir.dt.float8e4
AF = mybir.ActivationFunctionType
OP = mybir.AluOpType
DR = mybir.MatmulPerfMode.DoubleRow

B, N, D, S, HW = 2, 4096, 192, 16, 64
RD = D * S  # 3072
NT = 24  # channel tiles of 128
ROWS = 20  # slab rows per core (16 own + halo)
NL = ROWS * HW  # 1280 sites per core
SLAB0 = [0, 14, 30, 44]
OWN0 = [0, 2, 2, 4]

NF8 = np.dtype(ml_dtypes.float8_e4m3)
NBF = np.dtype(ml_dtypes.bfloat16)

_CACHE = {}
LAST = None


def _register_ntff_hook():
    """Register the axon NTFF profile hook if the image didn't inject it.

    concourse.bass_utils reads antenv.axon_hooks.get_axon_ntff_profile_hook()
    when trace=True under axon; this image's antenv lacks that module, so
    build the same ctypes hook trn_agent_boot would have registered.
    """
    import types
    import ctypes
    import contextlib

    if "antenv.axon_hooks" in sys.modules:
        return True
    try:
        import antenv
    except ImportError:
        return False
    so_path = "/opt/axon/libaxon_pjrt.so"
    if not os.path.exists(so_path):
        return False
    try:
        lib = ctypes.CDLL(so_path)
    except OSError:
        return False
    if not hasattr(lib, "axon_start_nrt_profile"):
        return False
    lib.axon_start_nrt_profile.argtypes = [
        ctypes.POINTER(ctypes.c_int64),
        ctypes.c_size_t,
    ]
    lib.axon_start_nrt_profile.restype = ctypes.c_int64
    lib.axon_stop_nrt_profile.argtypes = [ctypes.c_char_p]
    lib.axon_stop_nrt_profile.restype = ctypes.c_int64

    @contextlib.contextmanager
    def _hook(output_dir, device_ids):
        import jax

        jax.devices()
        if device_ids:
            ids = (ctypes.c_int64 * len(device_ids))(*device_ids)
            rc = lib.axon_start_nrt_profile(ids, len(device_ids))
        else:
            rc = lib.axon_start_nrt_profile(None, 0)
        if rc != 0:
            raise RuntimeError(f"axon_start_nrt_profile rc={rc}")
        try:
            yield
        finally:
            n = lib.axon_stop_nrt_profile(str(output_dir).encode())
            if n < 0:
                raise RuntimeError(f"axon_stop_nrt_profile rc={n}")

    mod = types.ModuleType("antenv.axon_hooks")
    _store = {"h": _hook}
    mod.set_axon_ntff_profile_hook = lambda h: _store.__setitem__("h", h)
    mod.get_axon_ntff_profile_hook = lambda: _store["h"]
    sys.modules["antenv.axon_hooks"] = mod
    antenv.axon_hooks = mod
    return True


def _softplus_np(v):
    return np.logaddexp(0.0, v)


def _build(K: int, inv_g: float, inv_p: float, sh: float, fast5: bool):
    dt = 1.0 / K if K > 0 else 1.0
    opt = fast5 and K == 2  # shrinking update regions + 5-point conv
    nc = bacc.Bacc(None, target_bir_lowering=False, debug=False)

    xcm_d = nc.dram_tensor("xcm", [D, NL], F32, kind="ExternalInput")
    h0b_d = nc.dram_tensor("h0b", [RD, NL], BF16, kind="ExternalInput")
    hf80_d = nc.dram_tensor("hf80", [RD, NL], FP8, kind="ExternalInput")
    p1a_d = nc.dram_tensor("p1a", [RD, NL], BF16, kind="ExternalInput")
    p1b_d = nc.dram_tensor("p1b", [RD, NL], BF16, kind="ExternalInput")
    ddb_d = nc.dram_tensor("ddb", [RD, NL], BF16, kind="ExternalInput")
    u1b_d = nc.dram_tensor("u1b", [RD, NL], BF16, kind="ExternalInput")
    cmb_d = nc.dram_tensor("cmb", [128, NL], BF16, kind="ExternalInput")
    dparam_d = nc.dram_tensor("dparam", [D, 1], F32, kind="ExternalInput")
    bg_d = nc.dram_tensor("bg", [RD, 1], F32, kind="ExternalInput")
    w9_d = nc.dram_tensor("w9", [RD, 9], F32, kind="ExternalInput")
    cb5_d = nc.dram_tensor("cb5", [RD, 1], F32, kind="ExternalInput")
    bd5_d = nc.dram_tensor("bd5", [RD, 1], F32, kind="ExternalInput")
    wg8_d = nc.dram_tensor("wg8", [RD, RD], FP8, kind="ExternalInput")
    wp8_d = nc.dram_tensor("wp8", [RD, RD], FP8, kind="ExternalInput")
    sely_d = nc.dram_tensor("sely", [128, NT * 128], BF16, kind="ExternalInput")
    y_d = nc.dram_tensor("y", [D, NL], F32, kind="ExternalOutput")

    NK2 = NT // 2  # DoubleRow k-pairs

    def chunks(ne):
        out, n0 = [], 0
        while n0 < ne:
            out.append((n0, min(512, ne - n0)))
            n0 += 512
        return out

    if opt:
        NE_S = [1216, 1152]  # rows 0..18 after step 1, rows 0..17 after step 2
        RE_S = [19, 18]
        NE_F = 1152
    else:
        NE_S = [NL] * max(K, 1)
        RE_S = [ROWS] * max(K, 1)
        NE_F = NL

    with tile.TileContext(nc) as tc:
        with tc.tile_pool(name="const", bufs=1) as const, \
             tc.tile_pool(name="hp", bufs=1) as hp, \
             tc.tile_pool(name="wsl", bufs=2) as wsl, \
             tc.tile_pool(name="bst", bufs=2) as bst, \
             tc.tile_pool(name="ust", bufs=2) as ust, \
             tc.tile_pool(name="work", bufs=1) as work, \
             tc.tile_pool(name="wk2", bufs=2) as wk2, \
             tc.tile_pool(name="psum", bufs=1, space="PSUM") as psum, \
             tc.tile_pool(name="ps2", bufs=2, space="PSUM") as ps2:

            # ---- constants first on the scalar queue (tiny) ----
            dpA = const.tile([128, 1], F32, tag="dpA")
            dpB = const.tile([64, 1], F32, tag="dpB")
            nc.scalar.dma_start(dpA[:], dparam_d[0:128, :])
            nc.scalar.dma_start(dpB[:], dparam_d[128:192, :])
            bg_sb = const.tile([128, NT], F32, tag="bg")
            nc.scalar.dma_start(bg_sb[:].rearrange("p (t o) -> p t o", o=1),
                                bg_d[:].rearrange("(t p) o -> p t o", p=128))
            if opt:
                cb5_sb = const.tile([128, NT], F32, tag="cb5")
                nc.scalar.dma_start(cb5_sb[:].rearrange("p (t o) -> p t o", o=1),
                                    cb5_d[:].rearrange("(t p) o -> p t o", p=128))
                bd5_sb = const.tile([128, NT], F32, tag="bd5")
                nc.scalar.dma_start(bd5_sb[:].rearrange("p (t o) -> p t o", o=1),
                                    bd5_d[:].rearrange("(t p) o -> p t o", p=128))
            else:
                w9_sb = const.tile([128, NT, 9], F32, tag="w9")
                nc.scalar.dma_start(w9_sb[:],
                                    w9_d[:].rearrange("(t p) j -> p t j", p=128))

            # ---- persistent state (hf8 on sync ahead of weights; hst on scalar) ----
            hst = hp.tile([128, NT, NL], BF16, tag="hst")
            hf8 = [hp.tile([128, NT, NL], FP8, tag=f"hf8{i}", name=f"hf8{i}")
                   for i in range(2)]
            if K > 0:
                nc.sync.dma_start(hf8[0][:],
                                  hf80_d[:].rearrange("(t p) n -> p t n", p=128))
            nc.scalar.dma_start(hst[:], h0b_d[:].rearrange("(t p) n -> p t n", p=128))
            cmb_sb = const.tile([128, NL], BF16, tag="cmb")
            nc.scalar.dma_start(cmb_sb[:], cmb_d[:])
            sely = const.tile([128, NT * 128], BF16, tag="sely")
            nc.scalar.dma_start(sely[:], sely_d[:])

            # ---- K integration steps ----
            for s in range(K):
                cur = hf8[s % 2]
                nxt = hf8[(s + 1) % 2]
                p1_d = p1a_d if s == 0 else p1b_d
                last = s == K - 1
                ne = NE_S[s]
                re = RE_S[s]
                nsp = chunks(ne)
                for rt in range(NT):
                    r0 = 128 * rt
                    wgt = wsl.tile([128, NT * 128], FP8, tag="wgt")
                    wpt = wsl.tile([128, NT * 128], FP8, tag="wpt")
                    nc.sync.dma_start(wgt[:], wg8_d[r0:r0 + 128, :])
                    nc.sync.dma_start(wpt[:], wp8_d[r0:r0 + 128, :])
                    wgt3 = wgt[:].rearrange("p (t m) -> p t m", m=128)
                    wpt3 = wpt[:].rearrange("p (t m) -> p t m", m=128)
                    p1t = bst.tile([128, NL], BF16, tag="p1t")
                    ddbt = bst.tile([128, NL], BF16, tag="ddbt")
                    nc.sync.dma_start(p1t[:, 0:ne], p1_d[r0:r0 + 128, 0:ne])
                    nc.sync.dma_start(ddbt[:, 0:ne], ddb_d[r0:r0 + 128, 0:ne])
                    if s > 0:
                        u1t = ust.tile([128, NL], BF16, tag="u1t")
                        nc.sync.dma_start(u1t[:, 0:ne], u1b_d[r0:r0 + 128, 0:ne])

                    tmp = wk2.tile([128, NL], F32, tag="tmp")
                    dh = work.tile([128, NL], BF16, tag="dh")

                    # f1 seed (+ state): tmp = hst * (dtA*dsb + 1)  [p1 host-folded]
                    nc.vector.tensor_tensor(tmp[:, 0:ne], hst[:, rt, 0:ne],
                                            p1t[:, 0:ne], OP.mult)
                    if s > 0:
                        nc.gpsimd.tensor_tensor(tmp[:, 0:ne], tmp[:, 0:ne],
                                                u1t[:, 0:ne], OP.add)

                    # gate matmuls (fp8 DoubleRow over 12 k-pairs)
                    pgs = [ps2.tile([128, 512], F32, tag="pg0", name="pg0"),
                           ps2.tile([128, 512], F32, tag="pg1", name="pg1"),
                           psum.tile([128, 512], F32, tag="pg2", name="pg2")]
                    pps = [psum.tile([128, 512], F32, tag=f"pp{j}", name=f"pp{j}")
                           for j in range(3)]
                    for kk in range(NK2):
                        for j, (n0, nw) in enumerate(nsp):
                            nc.tensor.matmul(pgs[j][:, 0:nw],
                                             wgt3[:, 2 * kk:2 * kk + 2, :],
                                             cur[:, 2 * kk:2 * kk + 2, n0:n0 + nw],
                                             start=(kk == 0), stop=(kk == NK2 - 1),
                                             perf_mode=DR)

                    hv = hst[:, rt, :].rearrange("p (r c) -> p r c", c=HW)
                    dv = dh[:].rearrange("p (r c) -> p r c", c=HW)
                    if opt:
                        # 5-point stencil: dh = (N+S+E+W) + (c/b)*C; b*dt folded
                        # into the f2 product below.
                        nc.vector.tensor_tensor(dv[:, 1:re, :], hv[:, 0:re - 1, :],
                                                hv[:, 2:re + 1, :], OP.add)
                        nc.vector.tensor_tensor(dv[:, 0:1, :], hv[:, 0:1, :],
                                                hv[:, 1:2, :], OP.add)
                        nc.vector.tensor_tensor(dv[:, 0:re, 1:HW], dv[:, 0:re, 1:HW],
                                                hv[:, 0:re, 0:HW - 1], OP.add)
                        nc.gpsimd.tensor_tensor(dv[:, 0:re, 0:1], dv[:, 0:re, 0:1],
                                                hv[:, 0:re, 0:1], OP.add)
                        nc.vector.tensor_tensor(dv[:, 0:re, 0:HW - 1],
                                                dv[:, 0:re, 0:HW - 1],
                                                hv[:, 0:re, 1:HW], OP.add)
                        nc.gpsimd.tensor_tensor(dv[:, 0:re, HW - 1:HW],
                                                dv[:, 0:re, HW - 1:HW],
                                                hv[:, 0:re, HW - 1:HW], OP.add)
                        nc.vector.scalar_tensor_tensor(dh[:, 0:ne], hst[:, rt, 0:ne],
                                                       cb5_sb[:, rt:rt + 1],
                                                       dh[:, 0:ne], OP.mult, OP.add)
                        # f2 = (dh * b * dt) * ddb
                        nc.vector.scalar_tensor_tensor(
                            dh[:, 0:ne], dh[:, 0:ne], bd5_sb[:, rt:rt + 1],
                            ddbt[:, 0:ne], OP.mult, OP.mult)
                    else:
                        # general depthwise 3x3 (dt folded into w9)
                        def segs(dd, n):
                            if dd == 0:
                                return [((0, n), (0, n))]
                            if dd == -1:
                                return [((1, n - 1), (0, n - 1)), ((0, 1), (0, 1))]
                            return [((0, n - 1), (1, n - 1)),
                                    ((n - 1, 1), (n - 1, 1))]

                        first = True
                        for di in (-1, 0, 1):
                            for dj in (-1, 0, 1):
                                idx = 3 * (di + 1) + (dj + 1)
                                w_s = w9_sb[:, rt, idx:idx + 1]
                                for (ro, rn), (ri, _) in segs(di, ROWS):
                                    for (co, cn), (ci, _) in segs(dj, HW):
                                        o = dv[:, ro:ro + rn, co:co + cn]
                                        i_ = hv[:, ri:ri + rn, ci:ci + cn]
                                        if first:
                                            nc.vector.tensor_scalar_mul(o, i_, w_s)
                                        else:
                                            nc.vector.scalar_tensor_tensor(
                                                o, i_, w_s, o, OP.mult, OP.add)
                                first = False
                        nc.vector.tensor_tensor(dh[:, 0:ne], dh[:, 0:ne],
                                                ddbt[:, 0:ne], OP.mult)

                    # sigmoid gate (descaled), overlaps the proj matmuls below
                    gates = []
                    for j, (n0, nw) in enumerate(nsp):
                        g = work.tile([128, 512], BF16, tag=f"gate{j}", name=f"gate{j}")
                        nc.scalar.activation(g[:, 0:nw], pgs[j][:, 0:nw], AF.Sigmoid,
                                             bias=bg_sb[:, rt:rt + 1], scale=inv_g)
                        gates.append(g)

                    # proj matmuls
                    for kk in range(NK2):
                        for j, (n0, nw) in enumerate(nsp):
                            nc.tensor.matmul(pps[j][:, 0:nw],
                                             wpt3[:, 2 * kk:2 * kk + 2, :],
                                             cur[:, 2 * kk:2 * kk + 2, n0:n0 + nw],
                                             start=(kk == 0), stop=(kk == NK2 - 1),
                                             perf_mode=DR)

                    # f3 = gate * proj (descaled, dt folded); tmp += f3; tmp += dh
                    reacts = []
                    for j, (n0, nw) in enumerate(nsp):
                        rc = work.tile([128, 512], BF16, tag=f"react{j}",
                                       name=f"react{j}")
                        nc.scalar.activation(rc[:, 0:nw], pps[j][:, 0:nw], AF.Copy,
                                             scale=dt * inv_p)
                        reacts.append(rc)
                    for j, (n0, nw) in enumerate(nsp):
                        f3c = work.tile([128, 512], F32, tag="f3c")
                        nc.vector.tensor_tensor(f3c[:, 0:nw], reacts[j][:, 0:nw],
                                                gates[j][:, 0:nw], OP.mult)
                        nc.vector.tensor_tensor(tmp[:, n0:n0 + nw], tmp[:, n0:n0 + nw],
                                                f3c[:, 0:nw], OP.add)
                    # final accumulate writes the bf16 state directly
                    nc.gpsimd.tensor_tensor(hst[:, rt, 0:ne], tmp[:, 0:ne],
                                            dh[:, 0:ne], OP.add)
                    if not last:
                        nc.vector.tensor_scalar_mul(nxt[:, rt, 0:ne],
                                                    hst[:, rt, 0:ne], sh)
                    else:
                        # fold the output z = h*Cm_bc in place (consumed by sely)
                        nc.vector.tensor_tensor(hst[:, rt, 0:NE_F],
                                                hst[:, rt, 0:NE_F],
                                                cmb_sb[:, 0:NE_F], OP.mult)

            # ---- final: y = sum_s h*Cm_bc + x*Dp ----
            nspf = chunks(NE_F)
            pys = [ps2.tile([128, 512], F32, tag="pg0", name="py0"),
                   ps2.tile([128, 512], F32, tag="pg1", name="py1"),
                   psum.tile([128, 512], F32, tag="pg2", name="py2")]
            pyB = [psum.tile([128, 512], F32, tag=f"pp{j}", name=f"pyB{j}")
                   for j in range(3)]
            for rt in range(NT):
                if K == 0:
                    nc.vector.tensor_tensor(hst[:, rt, 0:NE_F], hst[:, rt, 0:NE_F],
                                            cmb_sb[:, 0:NE_F], OP.mult)
                bank = pys if rt < 16 else pyB
                st = rt == 0 or rt == 16
                sp_ = rt == 15 or rt == NT - 1
                for j, (n0, nw) in enumerate(nspf):
                    nc.tensor.matmul(bank[j][:, 0:nw], sely[:, 128 * rt:128 * rt + 128],
                                     hst[:, rt, n0:n0 + nw], start=st, stop=sp_)
            for j, (n0, nw) in enumerate(nspf):
                xfA = work.tile([128, 512], F32, tag="f3c", name="xfA")
                nc.sync.dma_start(xfA[:, 0:nw], xcm_d[0:128, n0:n0 + nw])
                yA = work.tile([128, 512], F32, tag="yA", name=f"yA{j}")
                nc.vector.scalar_tensor_tensor(yA[:, 0:nw], xfA[:, 0:nw], dpA[:],
                                               pys[j][:, 0:nw], OP.mult, OP.add)
                nc.sync.dma_start(y_d[0:128, n0:n0 + nw], yA[:, 0:nw])
                xfB = work.tile([64, 512], F32, tag="xfB")
                nc.sync.dma_start(xfB[:, 0:nw], xcm_d[128:192, n0:n0 + nw])
                yB = work.tile([64, 512], F32, tag="yB")
                nc.vector.scalar_tensor_tensor(yB[:, 0:nw], xfB[:, 0:nw], dpB[:],
                                               pyB[j][0:64, 0:nw], OP.mult, OP.add)
                nc.sync.dma_start(y_d[128:192, n0:n0 + nw], yB[:, 0:nw])

    nc.compile()
    return nc


def _pow2_scale(target, amax):
    if amax <= 0:
        return 1.0
    return float(2.0 ** np.floor(np.log2(target / amax)))


def _prep(x, dt_self_W, dt_self_b, dt_diff_W, dt_diff_b, B_proj_W, C_proj_W,
          D_param, A_log, diff_conv_w, react_gate_W, react_gate_b,
          react_proj_W, dt):
    A = -_softplus_np(np.asarray(A_log, np.float32))          # (D, S)
    dtA1 = (dt * (A + 1.0)).reshape(RD, 1).astype(np.float32)
    dtA2 = (dt * A).reshape(RD, 1).astype(np.float32)
    w33 = np.asarray(diff_conv_w, np.float32)[:, 0]           # (D, 3, 3)
    w9 = (dt * w33).reshape(D, 1, 9)
    w9 = np.broadcast_to(w9, (D, S, 9)).reshape(RD, 9).astype(np.float32)
    w9f = (dt * w33[:, ::-1, :]).reshape(D, 1, 9)             # vertically flipped
    w9f = np.broadcast_to(w9f, (D, S, 9)).reshape(RD, 9).astype(np.float32)

    # 5-point stencil detection: corners zero, N==S==E==W per channel
    b5 = w33[:, 0, 1]
    fast5 = bool(
        np.all(w33[:, [0, 0, 2, 2], [0, 2, 0, 2]] == 0.0)
        and np.all(np.abs(w33[:, 1, 0] - b5) <= 1e-12)
        and np.all(np.abs(w33[:, 1, 2] - b5) <= 1e-12)
        and np.all(np.abs(w33[:, 2, 1] - b5) <= 1e-12)
        and np.all(np.abs(b5) > 1e-30)
    )
    if fast5:
        cb5 = (w33[:, 1, 1] / b5).astype(np.float32)
        bd5 = (dt * b5).astype(np.float32)
    else:
        cb5 = np.zeros(D, np.float32)
        bd5 = np.zeros(D, np.float32)
    cb5 = np.broadcast_to(cb5[:, None], (D, S)).reshape(RD, 1).copy()
    bd5 = np.broadcast_to(bd5[:, None], (D, S)).reshape(RD, 1).copy()

    WgT = np.ascontiguousarray(np.asarray(react_gate_W, np.float32).T)
    WpT = np.ascontiguousarray(np.asarray(react_proj_W, np.float32).T)
    sg = _pow2_scale(200.0, np.abs(WgT).max())
    sp = _pow2_scale(200.0, np.abs(WpT).max())

    x = np.asarray(x, np.float32)
    Bm = x @ np.asarray(B_proj_W, np.float32).T               # (B, N, S)
    Cm = x @ np.asarray(C_proj_W, np.float32).T               # (B, N, S)
    d_self = np.minimum(
        _softplus_np(x @ np.asarray(dt_self_W, np.float32).T
                     + np.asarray(dt_self_b, np.float32)), 0.15)
    d_diff = np.minimum(
        _softplus_np(x @ np.asarray(dt_diff_W, np.float32).T
                     + np.asarray(dt_diff_b, np.float32)), 0.15)
    maxh0 = (np.abs(x).max(-1) * np.abs(Bm).max(-1)).max()
    sh = _pow2_scale(200.0, 2.2 * maxh0)

    def tilemajor(WT, sc):
        a = WT.reshape(NT, 128, NT, 128).transpose(2, 1, 0, 3).reshape(RD, RD)
        return np.clip(a * sc, -240.0, 240.0).astype(NF8)

    sely = np.zeros((128, NT * 128), np.float32)
    for t in range(NT):
        for p in range(128):
            m = 8 * t + p // 16 if t < 16 else 8 * (t - 16) + p // 16
            sely[p, 128 * t + m] = 1.0

    shared = dict(
        dparam=np.asarray(D_param, np.float32).reshape(D, 1),
        bg=np.asarray(react_gate_b, np.float32).reshape(RD, 1),
        cb5=cb5,
        bd5=bd5,
        wg8=tilemajor(WgT, sg),
        wp8=tilemajor(WpT, sp),
        sely=sely.astype(NBF),
    )
    fields = dict(Bm=Bm, Cm=Cm, d_self=d_self, d_diff=d_diff,
                  dtA1=dtA1, dtA2=dtA2)
    return shared, fields, w9, w9f, sg, sp, sh, fast5


def kernel(x, dt_self_W, dt_self_b, dt_diff_W, dt_diff_b, B_proj_W, C_proj_W,
           D_param, A_log, diff_conv_w, react_gate_W, react_gate_b,
           react_proj_W, K_steps):
    from concourse.bass_utils import run_bass_kernel_spmd

    K = int(np.asarray(K_steps).item())
    dt = 1.0 / K if K > 0 else 1.0

    x = np.asarray(x, np.float32)
    shared, fields, w9, w9f, sg, sp, sh, fast5 = _prep(
        x, dt_self_W, dt_self_b, dt_diff_W, dt_diff_b, B_proj_W, C_proj_W,
        D_param, A_log, diff_conv_w, react_gate_W, react_gate_b,
        react_proj_W, dt)
    key = (K, sg, sp, sh, fast5)
    if key not in _CACHE:
        _CACHE[key] = _build(K, 1.0 / (sg * sh), 1.0 / (sp * sh), sh, fast5)
    nc = _CACHE[key]

    xg = x.reshape(B, HW, HW, D)
    Bg = fields["Bm"].reshape(B, HW, HW, S)
    Cg = fields["Cm"].reshape(B, HW, HW, S)
    dsg = fields["d_self"].reshape(B, HW, HW, D).astype(np.float32)
    ddg = fields["d_diff"].reshape(B, HW, HW, D).astype(np.float32)
    dtA1 = fields["dtA1"]
    dtA2 = fields["dtA2"]
    in_maps = []
    for core in range(8):
        b, rb = core // 4, core % 4
        if rb == 3:
            sl = np.s_[63:43:-1]  # reversed slab, own at rows 0..15
            w9c = w9f
        else:
            sl = np.s_[SLAB0[rb]:SLAB0[rb] + ROWS]
            w9c = w9
        slab = xg[b, sl].reshape(NL, D)
        bslab = np.asarray(Bg[b, sl], np.float32).reshape(NL, S)
        cslab = np.asarray(Cg[b, sl], np.float32).reshape(NL, S)
        dss = dsg[b, sl].reshape(NL, D)
        dds = ddg[b, sl].reshape(NL, D)
        h0 = np.ascontiguousarray(
            (slab[:, :, None] * bslab[:, None, :]).reshape(NL, RD).T)  # [RD, NL]
        dsb = np.ascontiguousarray(np.repeat(dss.T, S, axis=0))        # [RD, NL]
        ddb = np.ascontiguousarray(np.repeat(dds.T, S, axis=0))
        u1 = dt * dsb * h0
        p1a = dtA1 * dsb + 1.0
        p1b = dtA2 * dsb + 1.0
        in_maps.append(dict(
            shared,
            xcm=np.ascontiguousarray(slab.T),
            w9=w9c,
            h0b=h0.astype(NBF),
            hf80=np.clip(h0 * sh, -240.0, 240.0).astype(NF8),
            p1a=p1a.astype(NBF),
            p1b=p1b.astype(NBF),
            ddb=ddb.astype(NBF),
            u1b=u1.astype(NBF),
            cmb=np.ascontiguousarray(np.tile(cslab.T, (8, 1))).astype(NBF),
        ))

    trace_ok = False
    try:
        trace_ok = _register_ntff_hook()
    except Exception:
        trace_ok = False
    if trace_ok:
        try:
            r = run_bass_kernel_spmd(nc, in_maps, list(range(8)), trace=True)
        except Exception:
            r = run_bass_kernel_spmd(nc, in_maps, list(range(8)))
    else:
        r = run_bass_kernel_spmd(nc, in_maps, list(range(8)))
    global LAST
    LAST = r
    res = r.results
    y = np.empty((B, N, D), np.float32)
    for core in range(8):
        b, rb = core // 4, core % 4
        yc = res[core]["y"]
        if rb == 3:
            blk = yc.reshape(D, ROWS, HW)[:, 15::-1, :].reshape(D, 1024)
            y[b, 3 * 1024:4 * 1024, :] = blk.T
        else:
            o = OWN0[rb] * HW
            y[b, rb * 1024:(rb + 1) * 1024, :] = yc[:, o:o + 1024].T
    return y
